# revision 24
# baseline (speedup 1.0000x reference)
"""AnchorAttentionBatched Trainium2 kernel.

Data-parallel over the batch: 8 batch elements -> 8 NeuronCores. Each core:
gather 512 anchor rows -> LayerNorm -> QKV -> 8-head attention over anchors
-> output projection -> scatter rows into the (pre-zeroed) output.

Layouts on device (per core):
  x    [a, d]   gathered anchor rows (f32), LN stats along free dim
  xT   [d, a]   normalized+gamma/beta, bf16, feature-major (6 tiles [128,512])
  qT,kT [hd, a] head-aligned feature-major bf16 (8 tiles [96,512] each)
  v    [e, h*97+hd] anchor-major bf16 with a ones column per head (col 96)
  attnT [e, a]  exp(scores^T * scale) bf16 (no max subtraction: |scores|<~3)
  outT_h [97, a] psum; row 96 = softmax denominator per anchor (from the
                 ones column riding the AV matmul)
  out_rows [a, o] f32, scattered to DRAM by anchor index
"""

import numpy as np

import concourse.bass as bass
import concourse.mybir as mybir
import concourse.tile as tile
from concourse import bacc
from concourse.bass import IndirectOffsetOnAxis
from concourse.bass_utils import run_bass_kernel_spmd
from concourse.masks import make_identity
from concourse.tile import add_dep_helper

B, S, D = 8, 8192, 768
A = 512          # anchors
H, HD = 8, 96    # heads, head dim
P = 128
NA = A // P      # 4 anchor chunks
ND = D // P      # 6 feature chunks
SCALE = 1.0 / np.sqrt(HD)
F32 = mybir.dt.float32
BF16 = mybir.dt.bfloat16
I32 = mybir.dt.int32

ZERO_FILL = False  # runtime pre-zeros ExternalOutput buffers (documented
                   # contract in run_bass_kernel_spmd / bass2jax); flip to
                   # True to write the zeros from the device instead.


def _build(zero_fill=ZERO_FILL):
    nc = bacc.Bacc(None)
    hs = nc.declare_dram_parameter("hs", [S, D], F32, isOutput=False)
    idx = nc.declare_dram_parameter("idx", [A], I32, isOutput=False)
    wqT = nc.declare_dram_parameter("wqT", [D, D], BF16, isOutput=False)
    wkT = nc.declare_dram_parameter("wkT", [D, D], BF16, isOutput=False)
    wvT = nc.declare_dram_parameter("wvT", [D, D], BF16, isOutput=False)
    woT = nc.declare_dram_parameter("woT", [D, D], BF16, isOutput=False)
    bq = nc.declare_dram_parameter("bq", [D], F32, isOutput=False)
    bk = nc.declare_dram_parameter("bk", [D], F32, isOutput=False)
    bv = nc.declare_dram_parameter("bv", [D], F32, isOutput=False)
    bo = nc.declare_dram_parameter("bo", [D], F32, isOutput=False)
    lng = nc.declare_dram_parameter("lng", [D], F32, isOutput=False)
    lnb = nc.declare_dram_parameter("lnb", [D], F32, isOutput=False)
    out = nc.declare_dram_parameter("out", [S, D], F32, isOutput=True)

    def bcast_ap(ap, p=P):
        return bass.AP(tensor=ap.tensor, offset=ap.offset, ap=[[0, p]] + list(ap.ap))

    from contextlib import ExitStack
    with tile.TileContext(nc) as tc, ExitStack() as ctx:
        const = ctx.enter_context(tc.tile_pool(name="const", bufs=1))
        work = ctx.enter_context(tc.tile_pool(name="work", bufs=1))
        attn_p = ctx.enter_context(tc.tile_pool(name="attn", bufs=8))
        small = ctx.enter_context(tc.tile_pool(name="small", bufs=2))
        orow_p = ctx.enter_context(tc.tile_pool(name="orow", bufs=2))
        pA = ctx.enter_context(tc.tile_pool(name="pA", bufs=6, space="PSUM"))
        pB = ctx.enter_context(tc.tile_pool(name="pB", bufs=2, space="PSUM"))

        # ---- indices first (gathers depend on them), then gathers can
        # stream on the SWDGE queue while HWDGE loads weights ----
        idx_t = []
        for i in range(NA):
            it = const.tile([P, 1], I32, tag=f"idx{i}")
            nc.sync.dma_start(out=it[:], in_=idx[i * P:(i + 1) * P, None])
            idx_t.append(it)

        x_t = []
        for i in range(NA):
            xt = work.tile([P, D], F32, tag=f"x{i}")
            nc.gpsimd.indirect_dma_start(
                out=xt[:], out_offset=None,
                in_=hs[:], in_offset=IndirectOffsetOnAxis(ap=idx_t[i][:, :1], axis=0),
            )
            x_t.append(xt)

        # ---- constants / weights (bf16, HWDGE) ----
        lng_t = const.tile([P, ND], F32, tag="lng")
        lnb_t = const.tile([P, ND], F32, tag="lnb")
        bqh = const.tile([HD, H], F32, tag="bqh")
        bkh = const.tile([HD, H], F32, tag="bkh")
        for src, dst, pp in ((lng, lng_t, P), (lnb, lnb_t, P), (bq, bqh, HD), (bk, bkh, HD)):
            nc.sync.dma_start(
                out=dst[:], in_=src[:].rearrange("(c p) -> p c", p=pp),
            )
        bv_bc = const.tile([P, D], F32, tag="bv_bc")
        bo_bc = const.tile([P, D], F32, tag="bo_bc")
        nc.sync.dma_start(out=bv_bc[:], in_=bcast_ap(bv[:]))
        nc.sync.dma_start(out=bo_bc[:], in_=bcast_ap(bo[:]))
        eps_t = const.tile([P, 1], F32, tag="eps")
        nc.vector.memset(eps_t[:], 1e-5)
        ident = const.tile([P, P], BF16, tag="ident")
        make_identity(nc, ident[:])

        wq_b = const.tile([P, ND * D], BF16, tag="wq")
        wk_b = const.tile([P, ND * D], BF16, tag="wk")
        wv_b = const.tile([P, ND * D], BF16, tag="wv")
        wo_b = const.tile([HD, H * D], BF16, tag="wo")
        for wt, dst in ((wqT, wq_b), (wkT, wk_b), (wvT, wv_b)):
            nc.sync.dma_start(
                out=dst[:].rearrange("p (c o) -> p c o", c=ND),
                in_=wt[:].rearrange("(c p) o -> p c o", p=P),
            )
        nc.sync.dma_start(
            out=wo_b[:].rearrange("p (h o) -> p h o", h=H),
            in_=woT[:].rearrange("(h p) o -> p h o", p=HD),
        )

        # ---- optional zero-fill of the output ----
        zinsts = []
        if zero_fill:
            zt = const.tile([P, 4 * D], F32, tag="zeros")
            nc.vector.memset(zt[:], 0.0)
            ov = out[:].rearrange("(k p x) o -> k p x o", p=P, x=4)
            zsrc = zt[:].rearrange("p (x o) -> p x o", x=4)
            for k in range(16):
                zinsts.append(nc.sync.dma_start(out=ov[k], in_=zsrc))

        # ---- LayerNorm ----
        xn_t = []
        for i in range(NA):
            stats = small.tile([P, 3, 6], F32, tag="stats")
            xg = x_t[i][:].rearrange("p (s f) -> p s f", s=3)
            for s in range(3):
                nc.vector.bn_stats(out=stats[:, s, :], in_=xg[:, s, :])
            mv = small.tile([P, 2], F32, tag="mv")
            nc.vector.bn_aggr(out=mv[:], in_=stats[:])
            # rstd = exp(-0.5*ln(var+eps)): keeps every ACT func in the single
            # natural_log_exp_and_others table set (no Sqrt-set thrash).
            nc.scalar.activation(
                out=mv[:, 1:2], in_=mv[:, 1:2],
                func=mybir.ActivationFunctionType.Ln, bias=eps_t[:], scale=1.0,
            )
            nc.scalar.activation(
                out=mv[:, 1:2], in_=mv[:, 1:2],
                func=mybir.ActivationFunctionType.Exp, scale=-0.5,
            )
            xn = work.tile([P, D], BF16, tag=f"xn{i}")
            nc.vector.tensor_scalar(
                out=xn[:], in0=x_t[i][:],
                scalar1=mv[:, 0:1], scalar2=mv[:, 1:2],
                op0=mybir.AluOpType.subtract, op1=mybir.AluOpType.mult,
            )
            xn_t.append(xn)

        # Preload the exp table set on ScalarE after the last LN Sqrt so the
        # ~2.7us ACT_TABLE_LOAD overlaps the projection phase instead of
        # stalling the first attention exp.
        scratch1 = const.tile([1, 1], F32, tag="scratch1")
        nc.scalar.activation(
            out=scratch1[:], in_=eps_t[0:1, 0:1],
            func=mybir.ActivationFunctionType.Ln, scale=1.0,
        )
        nc.scalar.activation(
            out=scratch1[:], in_=eps_t[0:1, 0:1],
            func=mybir.ActivationFunctionType.Exp, scale=1.0,
        )

        # ---- transpose to feature-major + gamma/beta ----
        xT = []
        for c in range(ND):
            ps = pA.tile([P, A], BF16, tag="pA")
            for i in range(NA):
                nc.tensor.transpose(
                    out=ps[:, i * P:(i + 1) * P],
                    in_=xn_t[i][:, c * P:(c + 1) * P],
                    identity=ident[:],
                )
            xt = work.tile([P, A], BF16, tag=f"xT{c}")
            nc.vector.tensor_scalar(
                out=xt[:], in0=ps[:],
                scalar1=lng_t[:, c:c + 1], scalar2=lnb_t[:, c:c + 1],
                op0=mybir.AluOpType.mult, op1=mybir.AluOpType.add,
            )
            xT.append(xt)

        # ---- Q/K projections (head-aligned, feature-major) ----
        qT, kT = [], []
        for h in range(H):
            pq = pA.tile([HD, A], F32, tag="pA")
            for c in range(ND):
                nc.tensor.matmul(
                    out=pq[:], lhsT=wq_b[:, c * D + h * HD: c * D + (h + 1) * HD],
                    rhs=xT[c][:], start=(c == 0), stop=(c == ND - 1),
                )
            qt = work.tile([HD, A], BF16, tag=f"qT{h}")
            nc.scalar.activation(
                out=qt[:], in_=pq[:], func=mybir.ActivationFunctionType.Identity,
                bias=bqh[:, h:h + 1], scale=1.0,
            )
            qT.append(qt)
            pk = pA.tile([HD, A], F32, tag="pA")
            for c in range(ND):
                nc.tensor.matmul(
                    out=pk[:], lhsT=wk_b[:, c * D + h * HD: c * D + (h + 1) * HD],
                    rhs=xT[c][:], start=(c == 0), stop=(c == ND - 1),
                )
            kt = work.tile([HD, A], BF16, tag=f"kT{h}")
            nc.vector.tensor_scalar_add(out=kt[:], in0=pk[:], scalar1=bkh[:, h:h + 1])
            kT.append(kt)

        # ---- V (anchor-major, 97-strided heads with ones column) ----
        v_t = []
        for e in range(NA):
            vt = work.tile([P, H * (HD + 1)], BF16, tag=f"v{e}")
            vv = vt[:].rearrange("p (h x) -> p h x", h=H)
            for g in range(2):  # head groups 0-3, 4-7
                pv = pA.tile([P, 4 * HD], F32, tag="pA")
                for c in range(ND):
                    nc.tensor.matmul(
                        out=pv[:],
                        lhsT=xT[c][:, e * P:(e + 1) * P],
                        rhs=wv_b[:, c * D + g * 4 * HD: c * D + (g + 1) * 4 * HD],
                        start=(c == 0), stop=(c == ND - 1),
                    )
                nc.vector.tensor_tensor(
                    out=vv[:, 4 * g:4 * (g + 1), 0:HD],
                    in0=pv[:].rearrange("p (h x) -> p h x", h=4),
                    in1=bv_bc[:, g * 4 * HD:(g + 1) * 4 * HD].rearrange(
                        "p (h x) -> p h x", h=4),
                    op=mybir.AluOpType.add,
                )
            nc.vector.memset(vv[:, :, HD:HD + 1], 1.0)
            v_t.append(vt)

        # ---- attention per head ----
        outTn = []
        for h in range(H):
            at_e = []
            for e in range(NA):
                ps = pA.tile([P, A], F32, tag="pA")
                nc.tensor.matmul(
                    out=ps[:], lhsT=kT[h][:, e * P:(e + 1) * P], rhs=qT[h][:],
                    start=True, stop=True,
                )
                at = attn_p.tile([P, A], BF16, tag="attnT")
                nc.scalar.activation(
                    out=at[:], in_=ps[:],
                    func=mybir.ActivationFunctionType.Exp, scale=float(SCALE),
                )
                at_e.append(at)
            po = pB.tile([HD + 1, A], F32, tag="pB")
            for e in range(NA):
                nc.tensor.matmul(
                    out=po[:], lhsT=v_t[e][:, h * (HD + 1):(h + 1) * (HD + 1)],
                    rhs=at_e[e][:], start=(e == 0), stop=(e == NA - 1),
                )
            # 1/s = exp(-ln s) on ScalarE: Ln and Exp share one ACT table set,
            # and DVE's exact reciprocal is an 8-cycle/elem iterative divide
            # (~4.4us per head) we keep off the critical path this way.
            lns = small.tile([1, A], F32, tag="lns")
            nc.scalar.activation(
                out=lns[:], in_=po[HD:HD + 1, :],
                func=mybir.ActivationFunctionType.Ln, scale=1.0,
            )
            rec = small.tile([1, A], F32, tag="rec")
            nc.scalar.activation(
                out=rec[:], in_=lns[:],
                func=mybir.ActivationFunctionType.Exp, scale=-1.0,
            )
            bc = small.tile([HD, A], F32, tag="bc")
            nc.gpsimd.partition_broadcast(out_ap=bc[:], in_ap=rec[:])
            on = work.tile([HD, A], BF16, tag=f"outTn{h}")
            nc.vector.tensor_tensor(
                out=on[:], in0=po[0:HD, :], in1=bc[:], op=mybir.AluOpType.mult,
            )
            outTn.append(on)

        # ---- output projection + scatter ----
        for i in range(NA):
            pf1 = pA.tile([P, 512], F32, tag="pA")
            pf2 = pA.tile([P, 256], F32, tag="pA")
            for h in range(H):
                lt = outTn[h][:, i * P:(i + 1) * P]
                nc.tensor.matmul(
                    out=pf1[:], lhsT=lt, rhs=wo_b[:, h * D: h * D + 512],
                    start=(h == 0), stop=(h == H - 1),
                )
            for h in range(H):
                lt = outTn[h][:, i * P:(i + 1) * P]
                nc.tensor.matmul(
                    out=pf2[:], lhsT=lt, rhs=wo_b[:, h * D + 512: (h + 1) * D],
                    start=(h == 0), stop=(h == H - 1),
                )
            orow = orow_p.tile([P, D], F32, tag="orow")
            nc.vector.tensor_tensor(
                out=orow[:, 0:512], in0=pf1[:], in1=bo_bc[:, 0:512],
                op=mybir.AluOpType.add,
            )
            nc.vector.tensor_tensor(
                out=orow[:, 512:768], in0=pf2[:], in1=bo_bc[:, 512:768],
                op=mybir.AluOpType.add,
            )
            sc = nc.gpsimd.indirect_dma_start(
                out=out[:],
                out_offset=IndirectOffsetOnAxis(ap=idx_t[i][:, :1], axis=0),
                in_=orow[:], in_offset=None,
            )
            for z in zinsts:
                add_dep_helper(sc.ins, z.ins, reason="scatter after zero-fill")

    # Pin ScalarE to the one table set containing Ln+Exp+Identity so the
    # per-head Ln/Exp pairs don't thrash ACT_TABLE_LOADs (~2.7us each).
    # The pass records the set's index within the list it is given, so fix
    # the emitted ids up to the set's true act_info.json index afterwards.
    import concourse.bacc as bacc_mod
    from concourse.hw_specs import get_activation_tables
    full = get_activation_tables(nc.m.arch)
    keep = "natural_log_exp_and_others"
    true_id = list(full.keys()).index(keep)
    orig = bacc_mod.get_activation_tables
    bacc_mod.get_activation_tables = lambda arch: {keep: full[keep]}
    try:
        nc.finalize()
    finally:
        bacc_mod.get_activation_tables = orig
    n_fixed = 0
    for b in nc.main_func.blocks:
        for i in b.instructions:
            if type(i).__name__ == "InstLoadActFuncSet":
                i.act_func_set_id = true_id
                n_fixed += 1
    assert n_fixed >= 1
    return nc


_CACHED = {}


def _get_nc():
    if "nc" not in _CACHED:
        _CACHED["nc"] = _build()
    return _CACHED["nc"]


def kernel(hidden_states, anchor_indices, ln_g, ln_b,
           wq, bq, wk, bk, wv, bv, wo, bo, _trace=False):
    import ml_dtypes
    nc = _get_nc()
    f32 = np.float32
    bf16 = np.dtype(ml_dtypes.bfloat16)
    shared = dict(
        wqT=np.ascontiguousarray(wq.T).astype(bf16),
        wkT=np.ascontiguousarray(wk.T).astype(bf16),
        wvT=np.ascontiguousarray(wv.T).astype(bf16),
        woT=np.ascontiguousarray(wo.T).astype(bf16),
        bq=np.ascontiguousarray(bq, dtype=f32),
        bk=np.ascontiguousarray(bk, dtype=f32),
        bv=np.ascontiguousarray(bv, dtype=f32),
        bo=np.ascontiguousarray(bo, dtype=f32),
        lng=np.ascontiguousarray(ln_g, dtype=f32),
        lnb=np.ascontiguousarray(ln_b, dtype=f32),
    )
    in_maps = [
        dict(
            hs=np.ascontiguousarray(hidden_states[i], dtype=f32),
            idx=np.ascontiguousarray(anchor_indices[i], dtype=np.int32),
            **shared,
        )
        for i in range(B)
    ]
    res = run_bass_kernel_spmd(nc, in_maps, core_ids=list(range(B)), trace=_trace)
    outp = np.stack([res.results[i]["out"] for i in range(B)]).astype(f32)
    if _trace:
        return outp, res
    return outp


# revision 29
# speedup vs baseline: 1.2453x; 1.2453x over previous
"""AnchorAttentionBatched Trainium2 kernel.

Data-parallel over the batch: 8 batch elements -> 8 NeuronCores. Each core:
gather 512 anchor rows -> LayerNorm -> QKV -> 8-head attention over anchors
-> output projection -> scatter rows into the (pre-zeroed) output.

Layouts on device (per core):
  x    [a, d]   gathered anchor rows (f32), LN stats along free dim
  xT   [d, a]   normalized+gamma/beta, bf16, feature-major (6 tiles [128,512])
  qT,kT [hd, a] head-aligned feature-major bf16 (8 tiles [96,512] each)
  v    [e, h*97+hd] anchor-major bf16 with a ones column per head (col 96)
  attnT [e, a]  exp(scores^T * scale) bf16 (no max subtraction: |scores|<~3)
  outT_h [97, a] psum; row 96 = softmax denominator per anchor (from the
                 ones column riding the AV matmul)
  out_rows [a, o] f32, scattered to DRAM by anchor index
"""

import numpy as np

import concourse.bass as bass
import concourse.mybir as mybir
import concourse.tile as tile
from concourse import bacc
from concourse.bass import IndirectOffsetOnAxis
from concourse.bass_utils import run_bass_kernel_spmd
from concourse.masks import make_identity
from concourse.tile import add_dep_helper

B, S, D = 8, 8192, 768
A = 512          # anchors
H, HD = 8, 96    # heads, head dim
P = 128
NA = A // P      # 4 anchor chunks
ND = D // P      # 6 feature chunks
SCALE = 1.0 / np.sqrt(HD)
F32 = mybir.dt.float32
BF16 = mybir.dt.bfloat16
I32 = mybir.dt.int32

ZERO_FILL = False  # runtime pre-zeros ExternalOutput buffers (documented
                   # contract in run_bass_kernel_spmd / bass2jax); flip to
                   # True to write the zeros from the device instead.


def _build(zero_fill=ZERO_FILL):
    nc = bacc.Bacc(None)
    hs = nc.declare_dram_parameter("hs", [S, D], F32, isOutput=False)
    idx = nc.declare_dram_parameter("idx", [A], I32, isOutput=False)
    wqT = nc.declare_dram_parameter("wqT", [D, D], BF16, isOutput=False)
    wkT = nc.declare_dram_parameter("wkT", [D, D], BF16, isOutput=False)
    wvT = nc.declare_dram_parameter("wvT", [D, D], BF16, isOutput=False)
    woT = nc.declare_dram_parameter("woT", [D, D], BF16, isOutput=False)
    bq = nc.declare_dram_parameter("bq", [D], F32, isOutput=False)
    bk = nc.declare_dram_parameter("bk", [D], F32, isOutput=False)
    bv = nc.declare_dram_parameter("bv", [D], F32, isOutput=False)
    bo = nc.declare_dram_parameter("bo", [D], F32, isOutput=False)
    lng = nc.declare_dram_parameter("lng", [D], F32, isOutput=False)
    lnb = nc.declare_dram_parameter("lnb", [D], F32, isOutput=False)
    out = nc.declare_dram_parameter("out", [S, D], F32, isOutput=True)

    def bcast_ap(ap, p=P):
        return bass.AP(tensor=ap.tensor, offset=ap.offset, ap=[[0, p]] + list(ap.ap))

    from contextlib import ExitStack
    with tile.TileContext(nc) as tc, ExitStack() as ctx:
        const = ctx.enter_context(tc.tile_pool(name="const", bufs=1))
        work = ctx.enter_context(tc.tile_pool(name="work", bufs=1))
        attn_p = ctx.enter_context(tc.tile_pool(name="attn", bufs=8))
        qk_p = ctx.enter_context(tc.tile_pool(name="qk", bufs=3))
        small = ctx.enter_context(tc.tile_pool(name="small", bufs=2))
        orow_p = ctx.enter_context(tc.tile_pool(name="orow", bufs=2))
        pA = ctx.enter_context(tc.tile_pool(name="pA", bufs=6, space="PSUM"))
        pB = ctx.enter_context(tc.tile_pool(name="pB", bufs=2, space="PSUM"))

        # ---- indices first (gathers depend on them), then gathers can
        # stream on the SWDGE queue while HWDGE loads weights ----
        idx_t = []
        for i in range(NA):
            it = const.tile([P, 1], I32, tag=f"idx{i}")
            nc.sync.dma_start(out=it[:], in_=idx[i * P:(i + 1) * P, None])
            idx_t.append(it)

        x_t = []
        for i in range(NA):
            xt = work.tile([P, D], F32, tag=f"x{i}")
            nc.gpsimd.indirect_dma_start(
                out=xt[:], out_offset=None,
                in_=hs[:], in_offset=IndirectOffsetOnAxis(ap=idx_t[i][:, :1], axis=0),
            )
            x_t.append(xt)

        # ---- constants / weights (bf16) split across both HWDGE rings:
        # wv/wq/wk on the ACT ring, everything else on the SP ring ----
        wq_b = const.tile([P, ND * D], BF16, tag="wq")
        wk_b = const.tile([P, ND * D], BF16, tag="wk")
        wv_b = const.tile([P, ND * D], BF16, tag="wv")
        wo_b = const.tile([HD, H * D], BF16, tag="wo")
        for wt, dst in ((wvT, wv_b), (wqT, wq_b), (wkT, wk_b)):
            nc.scalar.dma_start(
                out=dst[:].rearrange("p (c o) -> p c o", c=ND),
                in_=wt[:].rearrange("(c p) o -> p c o", p=P),
            )
        nc.sync.dma_start(
            out=wo_b[:].rearrange("p (h o) -> p h o", h=H),
            in_=woT[:].rearrange("(h p) o -> p h o", p=HD),
        )
        lng_t = const.tile([P, ND], F32, tag="lng")
        lnb_t = const.tile([P, ND], F32, tag="lnb")
        bqh = const.tile([HD, H], F32, tag="bqh")
        bkh = const.tile([HD, H], F32, tag="bkh")
        for src, dst, pp in ((lng, lng_t, P), (lnb, lnb_t, P), (bq, bqh, HD), (bk, bkh, HD)):
            nc.sync.dma_start(
                out=dst[:], in_=src[:].rearrange("(c p) -> p c", p=pp),
            )
        bv_bc = const.tile([P, D], F32, tag="bv_bc")
        bo_bc = const.tile([P, D], F32, tag="bo_bc")
        nc.sync.dma_start(out=bv_bc[:], in_=bcast_ap(bv[:]))
        nc.sync.dma_start(out=bo_bc[:], in_=bcast_ap(bo[:]))
        eps_t = const.tile([P, 1], F32, tag="eps")
        nc.vector.memset(eps_t[:], 1e-5)
        ident = const.tile([P, P], BF16, tag="ident")
        make_identity(nc, ident[:])

        # ---- optional zero-fill of the output ----
        zinsts = []
        if zero_fill:
            zt = const.tile([P, 4 * D], F32, tag="zeros")
            nc.vector.memset(zt[:], 0.0)
            ov = out[:].rearrange("(k p x) o -> k p x o", p=P, x=4)
            zsrc = zt[:].rearrange("p (x o) -> p x o", x=4)
            for k in range(16):
                zinsts.append(nc.sync.dma_start(out=ov[k], in_=zsrc))

        # ---- LayerNorm ----
        xn_t = []
        for i in range(NA):
            stats = small.tile([P, 2, 6], F32, tag="stats")
            xg = x_t[i][:].rearrange("p (s f) -> p s f", s=2)
            for s in range(2):
                nc.vector.bn_stats(out=stats[:, s, :], in_=xg[:, s, :])
            mv = small.tile([P, 2], F32, tag="mv")
            nc.vector.bn_aggr(out=mv[:], in_=stats[:])
            # rstd = exp(-0.5*ln(var+eps)): keeps every ACT func in the single
            # natural_log_exp_and_others table set (no Sqrt-set thrash).
            nc.scalar.activation(
                out=mv[:, 1:2], in_=mv[:, 1:2],
                func=mybir.ActivationFunctionType.Ln, bias=eps_t[:], scale=1.0,
            )
            nc.scalar.activation(
                out=mv[:, 1:2], in_=mv[:, 1:2],
                func=mybir.ActivationFunctionType.Exp, scale=-0.5,
            )
            xn = work.tile([P, D], BF16, tag=f"xn{i}")
            nc.vector.tensor_scalar(
                out=xn[:], in0=x_t[i][:],
                scalar1=mv[:, 0:1], scalar2=mv[:, 1:2],
                op0=mybir.AluOpType.subtract, op1=mybir.AluOpType.mult,
            )
            xn_t.append(xn)

        # Preload the exp table set on ScalarE after the last LN Sqrt so the
        # ~2.7us ACT_TABLE_LOAD overlaps the projection phase instead of
        # stalling the first attention exp.
        scratch1 = const.tile([1, 1], F32, tag="scratch1")
        nc.scalar.activation(
            out=scratch1[:], in_=eps_t[0:1, 0:1],
            func=mybir.ActivationFunctionType.Ln, scale=1.0,
        )
        nc.scalar.activation(
            out=scratch1[:], in_=eps_t[0:1, 0:1],
            func=mybir.ActivationFunctionType.Exp, scale=1.0,
        )

        # ---- transpose to feature-major + gamma/beta ----
        xT = []
        for c in range(ND):
            ps = pA.tile([P, A], BF16, tag="pA")
            for i in range(NA):
                nc.tensor.transpose(
                    out=ps[:, i * P:(i + 1) * P],
                    in_=xn_t[i][:, c * P:(c + 1) * P],
                    identity=ident[:],
                )
            xt = work.tile([P, A], BF16, tag=f"xT{c}")
            nc.vector.tensor_scalar(
                out=xt[:], in0=ps[:],
                scalar1=lng_t[:, c:c + 1], scalar2=lnb_t[:, c:c + 1],
                op0=mybir.AluOpType.mult, op1=mybir.AluOpType.add,
            )
            xT.append(xt)

        # ---- V first (anchor-major, 97-strided heads with ones column) so
        # the per-head Q/K/attention pipeline below is self-contained ----
        v_t = []
        for e in range(NA):
            vt = work.tile([P, H * (HD + 1)], BF16, tag=f"v{e}")
            vv = vt[:].rearrange("p (h x) -> p h x", h=H)
            for g in range(2):  # head groups 0-3, 4-7
                pv = pA.tile([P, 4 * HD], F32, tag="pA")
                for c in range(ND):
                    nc.tensor.matmul(
                        out=pv[:],
                        lhsT=xT[c][:, e * P:(e + 1) * P],
                        rhs=wv_b[:, c * D + g * 4 * HD: c * D + (g + 1) * 4 * HD],
                        start=(c == 0), stop=(c == ND - 1),
                    )
                nc.vector.tensor_tensor(
                    out=vv[:, 4 * g:4 * (g + 1), 0:HD],
                    in0=pv[:].rearrange("p (h x) -> p h x", h=4),
                    in1=bv_bc[:, g * 4 * HD:(g + 1) * 4 * HD].rearrange(
                        "p (h x) -> p h x", h=4),
                    op=mybir.AluOpType.add,
                )
            nc.vector.memset(vv[:, :, HD:HD + 1], 1.0)
            v_t.append(vt)

        # ---- per head: Q/K projection then attention, interleaved so PE
        # projection work overlaps ScalarE exps of earlier heads ----
        outTn = []
        for h in range(H):
            pq = pA.tile([HD, A], F32, tag="pA")
            for c in range(ND):
                nc.tensor.matmul(
                    out=pq[:], lhsT=wq_b[:, c * D + h * HD: c * D + (h + 1) * HD],
                    rhs=xT[c][:], start=(c == 0), stop=(c == ND - 1),
                )
            qt = qk_p.tile([HD, A], BF16, tag="qT")
            nc.vector.tensor_scalar_add(out=qt[:], in0=pq[:], scalar1=bqh[:, h:h + 1])
            pk = pA.tile([HD, A], F32, tag="pA")
            for c in range(ND):
                nc.tensor.matmul(
                    out=pk[:], lhsT=wk_b[:, c * D + h * HD: c * D + (h + 1) * HD],
                    rhs=xT[c][:], start=(c == 0), stop=(c == ND - 1),
                )
            kt = qk_p.tile([HD, A], BF16, tag="kT")
            nc.vector.tensor_scalar_add(out=kt[:], in0=pk[:], scalar1=bkh[:, h:h + 1])

            at_e = []
            for e in range(NA):
                ps = pA.tile([P, A], F32, tag="pA")
                nc.tensor.matmul(
                    out=ps[:], lhsT=kt[:, e * P:(e + 1) * P], rhs=qt[:],
                    start=True, stop=True,
                )
                at = attn_p.tile([P, A], BF16, tag="attnT")
                nc.scalar.activation(
                    out=at[:], in_=ps[:],
                    func=mybir.ActivationFunctionType.Exp, scale=float(SCALE),
                )
                at_e.append(at)
            po = pB.tile([HD + 1, A], F32, tag="pB")
            for e in range(NA):
                nc.tensor.matmul(
                    out=po[:], lhsT=v_t[e][:, h * (HD + 1):(h + 1) * (HD + 1)],
                    rhs=at_e[e][:], start=(e == 0), stop=(e == NA - 1),
                )
            # 1/s = exp(-ln s) on ScalarE: Ln and Exp share one ACT table set,
            # and DVE's exact reciprocal is an 8-cycle/elem iterative divide
            # (~4.4us per head) we keep off the critical path this way.
            lns = small.tile([1, A], F32, tag="lns")
            nc.scalar.activation(
                out=lns[:], in_=po[HD:HD + 1, :],
                func=mybir.ActivationFunctionType.Ln, scale=1.0,
            )
            rec = small.tile([1, A], F32, tag="rec")
            nc.scalar.activation(
                out=rec[:], in_=lns[:],
                func=mybir.ActivationFunctionType.Exp, scale=-1.0,
            )
            bc = small.tile([HD, A], F32, tag="bc")
            nc.gpsimd.partition_broadcast(out_ap=bc[:], in_ap=rec[:])
            on = work.tile([HD, A], BF16, tag=f"outTn{h}")
            nc.vector.tensor_tensor(
                out=on[:], in0=po[0:HD, :], in1=bc[:], op=mybir.AluOpType.mult,
            )
            outTn.append(on)

        # ---- output projection + scatter ----
        for i in range(NA):
            pf1 = pA.tile([P, 512], F32, tag="pA")
            pf2 = pA.tile([P, 256], F32, tag="pA")
            for h in range(H):
                lt = outTn[h][:, i * P:(i + 1) * P]
                nc.tensor.matmul(
                    out=pf1[:], lhsT=lt, rhs=wo_b[:, h * D: h * D + 512],
                    start=(h == 0), stop=(h == H - 1),
                )
            for h in range(H):
                lt = outTn[h][:, i * P:(i + 1) * P]
                nc.tensor.matmul(
                    out=pf2[:], lhsT=lt, rhs=wo_b[:, h * D + 512: (h + 1) * D],
                    start=(h == 0), stop=(h == H - 1),
                )
            orow = orow_p.tile([P, D], F32, tag="orow")
            nc.vector.tensor_tensor(
                out=orow[:, 0:512], in0=pf1[:], in1=bo_bc[:, 0:512],
                op=mybir.AluOpType.add,
            )
            nc.vector.tensor_tensor(
                out=orow[:, 512:768], in0=pf2[:], in1=bo_bc[:, 512:768],
                op=mybir.AluOpType.add,
            )
            sc = nc.gpsimd.indirect_dma_start(
                out=out[:],
                out_offset=IndirectOffsetOnAxis(ap=idx_t[i][:, :1], axis=0),
                in_=orow[:], in_offset=None,
            )
            for z in zinsts:
                add_dep_helper(sc.ins, z.ins, reason="scatter after zero-fill")

    # Pin ScalarE to the one table set containing Ln+Exp+Identity so the
    # per-head Ln/Exp pairs don't thrash ACT_TABLE_LOADs (~2.7us each).
    # The pass records the set's index within the list it is given, so fix
    # the emitted ids up to the set's true act_info.json index afterwards.
    import concourse.bacc as bacc_mod
    from concourse.hw_specs import get_activation_tables
    full = get_activation_tables(nc.m.arch)
    keep = "natural_log_exp_and_others"
    true_id = list(full.keys()).index(keep)
    orig = bacc_mod.get_activation_tables
    bacc_mod.get_activation_tables = lambda arch: {keep: full[keep]}
    try:
        nc.finalize()
    finally:
        bacc_mod.get_activation_tables = orig
    n_fixed = 0
    for b in nc.main_func.blocks:
        for i in b.instructions:
            if type(i).__name__ == "InstLoadActFuncSet":
                i.act_func_set_id = true_id
                n_fixed += 1
    assert n_fixed >= 1
    return nc


_CACHED = {}


def _get_nc():
    if "nc" not in _CACHED:
        _CACHED["nc"] = _build()
    return _CACHED["nc"]


def kernel(hidden_states, anchor_indices, ln_g, ln_b,
           wq, bq, wk, bk, wv, bv, wo, bo, _trace=False):
    import ml_dtypes
    nc = _get_nc()
    f32 = np.float32
    bf16 = np.dtype(ml_dtypes.bfloat16)
    shared = dict(
        wqT=np.ascontiguousarray(wq.T).astype(bf16),
        wkT=np.ascontiguousarray(wk.T).astype(bf16),
        wvT=np.ascontiguousarray(wv.T).astype(bf16),
        woT=np.ascontiguousarray(wo.T).astype(bf16),
        bq=np.ascontiguousarray(bq, dtype=f32),
        bk=np.ascontiguousarray(bk, dtype=f32),
        bv=np.ascontiguousarray(bv, dtype=f32),
        bo=np.ascontiguousarray(bo, dtype=f32),
        lng=np.ascontiguousarray(ln_g, dtype=f32),
        lnb=np.ascontiguousarray(ln_b, dtype=f32),
    )
    in_maps = [
        dict(
            hs=np.ascontiguousarray(hidden_states[i], dtype=f32),
            idx=np.ascontiguousarray(anchor_indices[i], dtype=np.int32),
            **shared,
        )
        for i in range(B)
    ]
    res = run_bass_kernel_spmd(nc, in_maps, core_ids=list(range(B)), trace=_trace)
    outp = np.stack([res.results[i]["out"] for i in range(B)]).astype(f32)
    if _trace:
        return outp, res
    return outp
